# revision 10
# baseline (speedup 1.0000x reference)
# Multi-head attention on 8 Trainium2 NeuronCores.
#
# Sharding: 8 cores = 4 batches x 2 sequence-halves. Each core receives its
# batch's full x (2048 rows) with its own query-half permuted to the front,
# computes Q for its 1024 rows and K/V for all 2048 keys (softmax over keys is
# permutation invariant), and writes a [1024, 768] slice of the output. No
# collectives.
#
# Per-core pipeline (bf16 matmuls, fp32 accumulation):
#   xT   = transpose(x) via PE                       [768, 2048]
#   qT/kT = (x @ Wq/Wk)^T  via lhsT=W, rhs=xT        [768, 1024/2048]
#   V    = x @ Wv (natural layout, +ones column)     [2048, 12, 65]
#   per head h: S^T = K Q^T  -> exp (ScalarE, scale=1/8, no max subtraction;
#   scores are O(1) so exp cannot overflow fp32) -> P^T
#   O^T|denom = [V_h | 1]^T-style matmul with P^T    [65, 1024]
#   attnT = O^T * (1/denom broadcast across partitions via DMA)
#   y = attn @ W_out + b_out (lhsT=attnT, rhs=W_out)
import numpy as np

B, N, D = 4, 2048, 768
H, DH = 12, 64
SCALE = DH ** -0.5
NQ = N // 2          # query rows per core
KT = D // 128        # 6 contraction tiles over D
NKT = N // 128       # 16 key tiles
RT = N // 128        # 16 row tiles of x

_CACHE = {}


def _build():
    if "nc" in _CACHE:
        return _CACHE["nc"]

    from concourse import bacc
    import concourse.tile as tile
    import concourse.mybir as mybir

    F32 = mybir.dt.float32
    BF16 = mybir.dt.bfloat16
    AF = mybir.ActivationFunctionType

    nc = bacc.Bacc("TRN2", target_bir_lowering=False, debug=False,
                   num_devices=8)

    x = nc.dram_tensor("x", [N, D], F32, kind="ExternalInput").ap()
    wqkv = nc.dram_tensor("w_qkv", [D, 3 * D], F32, kind="ExternalInput").ap()
    wout = nc.dram_tensor("w_out", [D, D], F32, kind="ExternalInput").ap()
    bout = nc.dram_tensor("b_out", [1, D], F32, kind="ExternalInput").ap()
    ident = nc.dram_tensor("ident", [128, 128], F32, kind="ExternalInput").ap()
    y = nc.dram_tensor("y", [NQ, D], F32, kind="ExternalOutput").ap()

    with tile.TileContext(nc) as tc:
        with tc.tile_pool(name="const", bufs=1) as const, \
             tc.tile_pool(name="persist", bufs=1) as persist:
            ident_sb = const.tile([128, 128], F32)
            nc.sync.dma_start(out=ident_sb, in_=ident)
            ones64f = const.tile([1, 64], F32)
            nc.vector.memset(ones64f, 1.0)
            ones64 = const.tile([1, 64], mybir.dt.float32r)
            nc.vector.tensor_copy(out=ones64, in_=ones64f)
            bias_bc = const.tile([128, D], F32)
            nc.gpsimd.dma_start(out=bias_bc, in_=bout.to_broadcast((128, D)))

            wout_bf = persist.tile([128, KT, D], BF16)
            qT = persist.tile([128, KT, NQ], BF16)
            kTt = persist.tile([128, KT, N], BF16)
            Vn = persist.tile([128, NKT, H, DH + 1], BF16)
            attnT = persist.tile([128, KT, NQ], BF16)

            nc.vector.memset(Vn[:, :, :, DH:DH + 1], 1.0)

            # ---------------- phase 1: xT, qT, kT, V ----------------
            with tc.tile_pool(name="p1", bufs=1) as p1pool, \
                 tc.tile_pool(name="wstage", bufs=2) as wstage, \
                 tc.tile_pool(name="xstage", bufs=3) as xstage, \
                 tc.tile_pool(name="tpsum", bufs=2, space="PSUM") as tpsum, \
                 tc.tile_pool(name="qkpsum", bufs=4, space="PSUM") as qkp:
                wqkv_bf = p1pool.tile([128, KT, 3 * D], BF16)
                for j in range(KT):
                    ws = wstage.tile([128, 3 * D], F32, tag="ws")
                    nc.sync.dma_start(out=ws, in_=wqkv[j * 128:(j + 1) * 128, :])
                    nc.vector.tensor_copy(out=wqkv_bf[:, j, :], in_=ws)
                for j in range(KT):
                    ws2 = wstage.tile([128, D], F32, tag="ws")
                    nc.sync.dma_start(out=ws2, in_=wout[j * 128:(j + 1) * 128, :])
                    nc.vector.tensor_copy(out=wout_bf[:, j, :], in_=ws2)

                xT = p1pool.tile([128, KT, N], BF16)
                for rt in range(RT):
                    xs = xstage.tile([128, D], F32, tag="xs")
                    nc.sync.dma_start(out=xs, in_=x[rt * 128:(rt + 1) * 128, :])
                    for g in range(2):
                        tp = tpsum.tile([128, 3, 128], F32, tag="tp")
                        for jj in range(3):
                            j = g * 3 + jj
                            nc.tensor.transpose(tp[:, jj, :],
                                                xs[:, j * 128:(j + 1) * 128],
                                                ident_sb)
                        nc.vector.tensor_copy(
                            out=xT[:, g * 3:(g + 1) * 3, rt * 128:(rt + 1) * 128],
                            in_=tp)

                # qT (cols 0..768 of W_qkv, only our 1024 q rows)
                for ct in range(KT):
                    for rc in range(0, NQ, 512):
                        ps = qkp.tile([128, 512], F32, tag="qk")
                        for j in range(KT):
                            nc.tensor.matmul(
                                ps, wqkv_bf[:, j, ct * 128:(ct + 1) * 128],
                                xT[:, j, rc:rc + 512],
                                start=(j == 0), stop=(j == KT - 1))
                        nc.vector.tensor_copy(out=qT[:, ct, rc:rc + 512], in_=ps)
                # kT (cols 768..1536, all 2048 rows)
                for ct in range(KT):
                    for rc in range(0, N, 512):
                        ps = qkp.tile([128, 512], F32, tag="qk")
                        for j in range(KT):
                            nc.tensor.matmul(
                                ps, wqkv_bf[:, j, D + ct * 128:D + (ct + 1) * 128],
                                xT[:, j, rc:rc + 512],
                                start=(j == 0), stop=(j == KT - 1))
                        nc.vector.tensor_copy(out=kTt[:, ct, rc:rc + 512], in_=ps)
                # V natural [key, vcol] (cols 1536..2304)
                for rt in range(RT):
                    for (c0, cw) in ((0, 512), (512, 256)):
                        ps = qkp.tile([128, 512], F32, tag="qk")
                        for j in range(KT):
                            nc.tensor.matmul(
                                ps[:, :cw], xT[:, j, rt * 128:(rt + 1) * 128],
                                wqkv_bf[:, j, 2 * D + c0:2 * D + c0 + cw],
                                start=(j == 0), stop=(j == KT - 1))
                        nc.vector.tensor_copy(
                            out=Vn[:, rt, c0 // DH:(c0 + cw) // DH, 0:DH],
                            in_=ps[:, :cw].rearrange("p (h d) -> p h d", d=DH))

            # ---------------- phase 2: attention ----------------
            with tc.tile_pool(name="spsum", bufs=2, space="PSUM") as spsum, \
                 tc.tile_pool(name="pvpsum", bufs=2, space="PSUM") as pvpsum, \
                 tc.tile_pool(name="ppool", bufs=4) as ppool, \
                 tc.tile_pool(name="rpool", bufs=2) as rpool:
                F32R = mybir.dt.float32r
                for h in range(H):
                    tj, po = divmod(h, 2)
                    po *= 64
                    pv = pvpsum.tile([DH + 1, NQ], mybir.dt.float32, tag="pv")
                    for kt in range(NKT):
                        sp = spsum.tile([128, NQ], mybir.dt.float32, tag="sp")
                        for rc in range(0, NQ, 512):
                            nc.tensor.matmul(
                                sp[:, rc:rc + 512],
                                kTt[po:po + 64, tj, kt * 128:(kt + 1) * 128],
                                qT[po:po + 64, tj, rc:rc + 512],
                                start=True, stop=True)
                        pt = ppool.tile([128, NQ], BF16, tag="pt")
                        nc.scalar.activation(pt, sp, AF.Exp, 0.0, SCALE)
                        for rc in range(0, NQ, 512):
                            nc.tensor.matmul(
                                pv[:, rc:rc + 512], Vn[:, kt, h, :],
                                pt[:, rc:rc + 512],
                                start=(kt == 0), stop=(kt == NKT - 1))
                    rcp = rpool.tile([1, NQ], F32R, tag="rcp")
                    with nc.allow_low_precision(reason="fp32r 1/denom feed"):
                        nc.vector.reciprocal(rcp, pv[DH:DH + 1, :])
                    # broadcast 1/denom across 64 partitions with a K=1 matmul
                    rb = spsum.tile([64, NQ], mybir.dt.float32, tag="sp")
                    for rc in range(0, NQ, 512):
                        nc.tensor.matmul(rb[:, rc:rc + 512],
                                         ones64,
                                         rcp[:, rc:rc + 512],
                                         start=True, stop=True)
                    rbs = rpool.tile([64, NQ], mybir.dt.float32, tag="rbs")
                    nc.vector.tensor_copy(out=rbs, in_=rb)
                    nc.vector.tensor_mul(attnT[po:po + 64, tj, :],
                                         pv[0:DH, :], rbs)

            # ---------------- phase 3: output projection ----------------
            with tc.tile_pool(name="ypsum", bufs=2, space="PSUM") as ypsum, \
                 tc.tile_pool(name="ypool", bufs=3) as ypool:
                for rt in range(NQ // 128):
                    yp = ypsum.tile([128, D], mybir.dt.float32, tag="yp")
                    for j in range(KT):
                        for (c0, cw) in ((0, 512), (512, 256)):
                            nc.tensor.matmul(
                                yp[:, c0:c0 + cw],
                                attnT[:, j, rt * 128:(rt + 1) * 128],
                                wout_bf[:, j, c0:c0 + cw],
                                start=(j == 0), stop=(j == KT - 1))
                    ys = ypool.tile([128, D], mybir.dt.float32, tag="ys")
                    nc.vector.tensor_add(ys, yp, bias_bc)
                    nc.sync.dma_start(out=y[rt * 128:(rt + 1) * 128, :], in_=ys)

    nc.compile()
    _CACHE["nc"] = nc
    return nc


def _in_maps(x, W_qkv, W_out, b_out):
    x = np.ascontiguousarray(np.asarray(x, dtype=np.float32))
    W_qkv = np.ascontiguousarray(np.asarray(W_qkv, dtype=np.float32))
    W_out = np.ascontiguousarray(np.asarray(W_out, dtype=np.float32))
    b_out = np.ascontiguousarray(np.asarray(b_out, dtype=np.float32)).reshape(1, D)
    ident = np.eye(128, dtype=np.float32)
    maps = []
    for c in range(8):
        b, half = divmod(c, 2)
        xb = x[b]
        xr = np.concatenate(
            [xb[half * NQ:(half + 1) * NQ], xb[(1 - half) * NQ:(2 - half) * NQ]],
            axis=0)
        maps.append({"x": np.ascontiguousarray(xr), "w_qkv": W_qkv,
                     "w_out": W_out, "b_out": b_out, "ident": ident})
    return maps


def kernel(x, W_qkv, W_out, b_out):
    from concourse import bass_utils
    nc = _build()
    maps = _in_maps(x, W_qkv, W_out, b_out)
    res = bass_utils.run_bass_kernel_spmd(nc, maps, core_ids=list(range(8)))
    out = np.empty((B, N, D), dtype=np.float32)
    for c in range(8):
        b, half = divmod(c, 2)
        out[b, half * NQ:(half + 1) * NQ] = res.results[c]["y"]
    return out


# revision 20
# speedup vs baseline: 9131.8633x; 9131.8633x over previous
# Multi-head attention on 8 Trainium2 NeuronCores.
#
# Sharding: 8 cores = 4 batches x 2 sequence-halves. Each core receives its
# batch's full x (2048 rows) with its own query-half permuted to the front,
# computes Q for its 1024 rows and K/V for all 2048 keys (softmax over keys is
# permutation invariant), and writes a [1024, 768] slice of the output. No
# collectives.
#
# Per-core pipeline (bf16 matmuls, fp32 accumulation):
#   xT   = transpose(x) via PE                       [768, 2048]
#   qT/kT = (x @ Wq/Wk)^T  via lhsT=W, rhs=xT        [768, 1024/2048]
#   V    = x @ Wv (natural layout, +ones column)     [2048, 12, 65]
#   per head h: S^T = K Q^T  -> exp (ScalarE, scale=1/8, no max subtraction;
#   scores are O(1) so exp cannot overflow fp32) -> P^T staged in SBUF
#   O^T|denom = [V_h | 1] matmul with P^T            [65, 1024]
#   attnT = O^T * (1/denom broadcast via K=1 fp32r matmul)
#   y = attn @ W_out + b_out (lhsT=attnT, rhs=W_out)
#
# Head h's score/exp stage runs interleaved with head h-1's PV stage, so the
# PE->ACT->PE dependency chain has a full head of slack and never stalls the
# in-order engines.
import numpy as np

B, N, D = 4, 2048, 768
H, DH = 12, 64
SCALE = DH ** -0.5
NQ = N // 2          # query rows per core
KT = D // 128        # 6 contraction tiles over D
NKT = N // 128       # 16 key tiles
RT = N // 128        # 16 row tiles of x

_CACHE = {}


def _build(reps=1, variant="full"):
    if ("nc", reps, variant) in _CACHE:
        return _CACHE[("nc", reps, variant)]

    from concourse import bacc
    import concourse.tile as tile
    import concourse.mybir as mybir

    F32 = mybir.dt.float32
    F32R = mybir.dt.float32r
    BF16 = mybir.dt.bfloat16
    AF = mybir.ActivationFunctionType

    nc = bacc.Bacc("TRN2", target_bir_lowering=False, debug=False,
                   num_devices=8)

    x = nc.dram_tensor("x", [N, D], F32, kind="ExternalInput").ap()
    wqkv = nc.dram_tensor("w_qkv", [D, 3 * D], F32, kind="ExternalInput").ap()
    wout = nc.dram_tensor("w_out", [D, D], F32, kind="ExternalInput").ap()
    bout = nc.dram_tensor("b_out", [1, D], F32, kind="ExternalInput").ap()
    ident = nc.dram_tensor("ident", [128, 128], F32, kind="ExternalInput").ap()
    y = nc.dram_tensor("y", [NQ, D], F32, kind="ExternalOutput").ap()

    with tile.TileContext(nc) as tc:
      for _rep in range(reps):
        with tc.tile_pool(name="const", bufs=1) as const, \
             tc.tile_pool(name="persist", bufs=1) as persist:

            ident_sb = const.tile([128, 128], F32)
            nc.sync.dma_start(out=ident_sb, in_=ident)
            bias_bc = const.tile([128, D], F32)
            nc.gpsimd.dma_start(out=bias_bc, in_=bout.to_broadcast((128, D)))

            wout_bf = persist.tile([128, KT, D], BF16)
            qT = persist.tile([128, KT, NQ], BF16)
            kTt = persist.tile([128, KT, N], BF16)
            Vn = persist.tile([128, NKT, H, 128], BF16)
            attnT = persist.tile([128, KT, NQ], BF16)

            nc.vector.memset(Vn[:, :, :, DH:], 1.0)

            # ------------- phase 1: weights, xT, qT, kT, V -------------
            with tc.tile_pool(name="p1", bufs=1) as p1pool, \
                 tc.tile_pool(name="wstage", bufs=1) as wstage, \
                 tc.tile_pool(name="xstage", bufs=2) as xstage, \
                 tc.tile_pool(name="qkp", bufs=4, space="PSUM") as qkp:
                wqkv_bf = p1pool.tile([128, KT, 3 * D], BF16)
                for j in range(KT):
                    ws = wstage.tile([128, 3 * D], F32, tag="ws", name=f"ws{j}")
                    nc.sync.dma_start(out=ws, in_=wqkv[j * 128:(j + 1) * 128, :])
                    nc.gpsimd.tensor_copy(out=wqkv_bf[:, j, :], in_=ws)
                for j in range(KT):
                    ws2 = wstage.tile([128, D], F32, tag="ws", name=f"wo{j}")
                    nc.sync.dma_start(out=ws2, in_=wout[j * 128:(j + 1) * 128, :])
                    nc.gpsimd.tensor_copy(out=wout_bf[:, j, :], in_=ws2)

                xT = p1pool.tile([128, KT, N], BF16)
                for rt in range(RT):
                    xs = xstage.tile([128, D], F32, tag="xs", name=f"xs{rt}")
                    nc.sync.dma_start(out=xs, in_=x[rt * 128:(rt + 1) * 128, :])
                    for g in range(2):
                        tp = qkp.tile([128, 3, 128], F32, tag="qk",
                                      name=f"tp{rt}_{g}")
                        for jj in range(3):
                            j = g * 3 + jj
                            nc.tensor.transpose(tp[:, jj, :],
                                                xs[:, j * 128:(j + 1) * 128],
                                                ident_sb)
                        nc.vector.tensor_copy(
                            out=xT[:, g * 3:(g + 1) * 3,
                                   rt * 128:(rt + 1) * 128],
                            in_=tp)

                def emit_qkvT(dst, wcol0, ct, rc):
                    ps = qkp.tile([128, 512], F32, tag="qk",
                                  name=f"qk{wcol0}_{ct}_{rc}")
                    c0 = wcol0 + ct * 128
                    for j in range(KT):
                        nc.tensor.matmul(ps, wqkv_bf[:, j, c0:c0 + 128],
                                         xT[:, j, rc:rc + 512],
                                         start=(j == 0), stop=(j == KT - 1))
                    nc.vector.tensor_copy(out=dst[:, ct, rc:rc + 512], in_=ps)

                for ct in range(KT):
                    for rc in range(0, NQ, 512):
                        emit_qkvT(qT, 0, ct, rc)
                    for rc in range(0, N, 512):
                        emit_qkvT(kTt, D, ct, rc)
                for rt in range(RT):
                    for (c0, cw) in ((0, 512), (512, 256)):
                        ps = qkp.tile([128, 512], F32, tag="qk",
                                      name=f"v{rt}_{c0}")
                        for j in range(KT):
                            nc.tensor.matmul(
                                ps[:, :cw],
                                xT[:, j, rt * 128:(rt + 1) * 128],
                                wqkv_bf[:, j, 2 * D + c0:2 * D + c0 + cw],
                                start=(j == 0), stop=(j == KT - 1))
                        nc.vector.tensor_copy(
                            out=Vn[:, rt, c0 // DH:(c0 + cw) // DH, 0:DH],
                            in_=ps[:, :cw].rearrange("p (h d) -> p h d", d=DH))

            # ------------- phase 2: attention (head-lagged pipeline) ----
            with tc.tile_pool(name="ppool", bufs=2) as ppool, \
                 tc.tile_pool(name="rpool", bufs=2) as rpool, \
                 tc.tile_pool(name="ypool", bufs=2) as ypool, \
                 tc.tile_pool(name="spsum", bufs=2, space="PSUM") as spsum, \
                 tc.tile_pool(name="pvpsum", bufs=2, space="PSUM") as pvpsum:

                pts = {}
                pvs = {}
                ptfix = None
                if variant in ("nx1", "nx2", "pvonly", "pvna", "pv64"):
                    ptfix = ppool.tile([128, NKT, NQ], BF16, tag="ptfix",
                                       bufs=1)
                    nc.vector.memset(ptfix[:, 0, :], 0.001)
                    for kk in range(1, NKT):
                        nc.vector.tensor_copy(out=ptfix[:, kk, :],
                                              in_=ptfix[:, 0, :])

                def emit_s(h, kt):
                    tj, po = divmod(h, 2)
                    po *= 64
                    if kt == 0:
                        pts[h] = ppool.tile([128, NKT, NQ], BF16, tag="pt",
                                            name=f"pt{h}")
                    sp = spsum.tile([128, NQ], F32, tag="sp",
                                    name=f"sp{h}_{kt}")
                    for rc in range(0, NQ, 512):
                        nc.tensor.matmul(
                            sp[:, rc:rc + 512],
                            kTt[po:po + 64, tj, kt * 128:(kt + 1) * 128],
                            qT[po:po + 64, tj, rc:rc + 512],
                            start=True, stop=True)
                    if variant == "dvexp":
                        nc.vector.tensor_copy(out=pts[h][:, kt, :], in_=sp)
                    elif variant in ("nx1", "sonly"):
                        pass
                    elif variant == "nx2":
                        nc.scalar.activation(pts[h][:, kt, :], sp, AF.Exp,
                                             0.0, SCALE)
                    else:
                        nc.scalar.activation(pts[h][:, kt, :], sp, AF.Exp,
                                             0.0, SCALE)

                def emit_pv(h, kt):
                    tj, po = divmod(h, 2)
                    po *= 64
                    if kt == 0:
                        pvs[h] = pvpsum.tile([128, NQ], F32, tag="pv",
                                             name=f"pv{h}")
                    pv = pvs[h]
                    ptsrc = (ptfix if variant in ("nx1", "nx2", "pvonly")
                             else pts[h])
                    for rc in range(0, NQ, 512):
                        nc.tensor.matmul(
                            pv[:, rc:rc + 512], Vn[:, kt, h, :],
                            ptsrc[:, kt, rc:rc + 512],
                            start=(kt == 0), stop=(kt == NKT - 1))
                    if kt == NKT - 1:
                        pts.pop(h, None)
                        rcp = rpool.tile([64, NQ], F32, tag="rcp",
                                         name=f"rcp{h}")
                        nc.vector.reciprocal(rcp, pv[DH:DH + 64, :])
                        nc.vector.tensor_mul(attnT[po:po + 64, tj, :],
                                             pv[0:DH, :], rcp)
                        del pvs[h]

                if variant == "noattn":
                    nc.vector.memset(attnT, 0.0)
                elif variant == "sonly":
                    for h in range(H):
                        for kt in range(NKT):
                            emit_s(h, kt)
                        del pts[h]
                    nc.vector.memset(attnT, 0.0)
                elif variant == "pvonly":
                    nc.vector.memset(attnT, 0.0)
                    for h in range(H):
                        for kt in range(NKT):
                            emit_pv(h, kt)
                elif variant == "pvna":
                    # PV matmuls without accumulation chains: independent
                    # start/stop into rotating sp slots
                    nc.vector.memset(attnT, 0.0)
                    for h in range(H):
                        for kt in range(NKT):
                            spx = spsum.tile([128, NQ], F32, tag="sp",
                                             name=f"spx{h}_{kt}")
                            for rc in range(0, NQ, 512):
                                nc.tensor.matmul(
                                    spx[:, rc:rc + 512],
                                    Vn[:, kt, h, :],
                                    ptfix[:, kt, rc:rc + 512],
                                    start=True, stop=True)
                elif variant == "pv64":
                    # accumulating PV with 64-col stationary (no ones col)
                    nc.vector.memset(attnT, 0.0)
                    for h in range(H):
                        pvx = pvpsum.tile([128, NQ], F32, tag="pv",
                                          name=f"pvx{h}")
                        for kt in range(NKT):
                            for rc in range(0, NQ, 512):
                                nc.tensor.matmul(
                                    pvx[0:DH, rc:rc + 512],
                                    Vn[:, kt, h, 0:DH],
                                    ptfix[:, kt, rc:rc + 512],
                                    start=(kt == 0), stop=(kt == NKT - 1))
                else:
                    for h in range(H):
                        for kt in range(NKT):
                            emit_s(h, kt)
                            if h >= 1:
                                emit_pv(h - 1, kt)
                    for kt in range(NKT):
                        emit_pv(H - 1, kt)

                # ------------- phase 3: output projection -------------
                for rt in range(NQ // 128):
                    yp = spsum.tile([128, D], F32, tag="sp", name=f"yp{rt}")
                    for j in range(KT):
                        for (c0, cw) in ((0, 512), (512, 256)):
                            nc.tensor.matmul(
                                yp[:, c0:c0 + cw],
                                attnT[:, j, rt * 128:(rt + 1) * 128],
                                wout_bf[:, j, c0:c0 + cw],
                                start=(j == 0), stop=(j == KT - 1))
                    ys = ypool.tile([128, D], F32, tag="ys", name=f"ys{rt}")
                    nc.vector.tensor_add(ys, yp, bias_bc)
                    nc.sync.dma_start(out=y[rt * 128:(rt + 1) * 128, :],
                                      in_=ys)

    nc.compile()
    _CACHE[("nc", reps, variant)] = nc
    return nc


def _in_maps(x, W_qkv, W_out, b_out):
    x = np.ascontiguousarray(np.asarray(x, dtype=np.float32))
    W_qkv = np.ascontiguousarray(np.asarray(W_qkv, dtype=np.float32))
    W_out = np.ascontiguousarray(np.asarray(W_out, dtype=np.float32))
    b_out = np.ascontiguousarray(np.asarray(b_out, dtype=np.float32)).reshape(1, D)
    ident = np.eye(128, dtype=np.float32)
    maps = []
    for c in range(8):
        b, half = divmod(c, 2)
        xb = x[b]
        xr = np.concatenate(
            [xb[half * NQ:(half + 1) * NQ], xb[(1 - half) * NQ:(2 - half) * NQ]],
            axis=0)
        maps.append({"x": np.ascontiguousarray(xr), "w_qkv": W_qkv,
                     "w_out": W_out, "b_out": b_out, "ident": ident})
    return maps


def kernel(x, W_qkv, W_out, b_out):
    from concourse import bass_utils
    nc = _build()
    maps = _in_maps(x, W_qkv, W_out, b_out)
    res = bass_utils.run_bass_kernel_spmd(nc, maps, core_ids=list(range(8)))
    out = np.empty((B, N, D), dtype=np.float32)
    for c in range(8):
        b, half = divmod(c, 2)
        out[b, half * NQ:(half + 1) * NQ] = res.results[c]["y"]
    return out
